# revision 1
# baseline (speedup 1.0000x reference)
"""AttentionLSTM Trainium2 kernel: 8-core tensor-parallel over the 4H gate dim.

Math per step t (reference):
    scores = (h @ A_flat) / 32         # per-sample: [N,L]
    w = softmax(scores)
    attn = A_flat @ w                  # [N,H]
    a = x_t@Wx + h@Wh + attn@Wattn + b # [N,4H]
    i,f,o,g = split(a); c = sig(f)*c + sig(i)*tanh(g); h = sig(o)*tanh(c)

Sharding: core k owns h-columns [128k,128k+128) and computes the 4 gate
strips for those columns (512 of 4096 gate cols). c stays sharded. Per
step one AllGather shares each core's transposed h-chunk + partial
scores. attn@Wattn is restructured as sum_l w_l * B_l with
B_l = A_flat[:,:,l] @ Wattn (built on device in a prologue); the
weighted sum runs on the PE as 16 PSUM-accumulating matmuls with
diag(w_l) stationary ("diag trick"). x@Wx is precomputed on device.
"""

import os
import sys

sys.path.insert(0, "/opt/trn_rl_repo")

import numpy as np

import concourse.bass as bass
import concourse.tile as tile
from concourse import bacc, mybir
from concourse.bass_utils import run_bass_kernel_spmd

N, T, D, H = 128, 64, 1024, 1024
L = 16
NC = 8
HCK = H // NC          # h-cols per core = 128
GC = 4 * HCK           # gate cols per core = 512
KC = 8                 # 128-row contraction chunks in D/H
P = 128

F32 = mybir.dt.float32
F32R = mybir.dt.float32r

_cache = {}


def _build(t_steps: int, use_cc: bool = True, repeat: int = 1):
    nc = bacc.Bacc(
        "TRN2",
        target_bir_lowering=False,
        debug=False,
        enable_asserts=False,
        num_devices=NC,
    )

    # ---- kernel I/O (per-core feeds prepared on host) ----
    # xT/at are sharded by row-chunk per core and all-gathered on device
    # (replicating them in the feed costs ~300 MB of host->device traffic).
    xTs = nc.dram_tensor("xTs", [P, T * P], F32R, kind="ExternalInput")
    wx = nc.dram_tensor("wx", [D, GC], F32R, kind="ExternalInput")
    wh = nc.dram_tensor("wh", [H, GC], F32R, kind="ExternalInput")
    wat = nc.dram_tensor("wat", [H, GC], F32R, kind="ExternalInput")
    bia = nc.dram_tensor("bia", [P, GC], F32R, kind="ExternalInput")
    asc = nc.dram_tensor("asc", [P, L * HCK], F32, kind="ExternalInput")  # [n,l,hc]/32
    ats = nc.dram_tensor("ats", [P, L * P], F32R, kind="ExternalInput")  # [h-chunk, l, n]
    eyeT = nc.dram_tensor("eyeT", [P, P], F32R, kind="ExternalInput")
    out = nc.dram_tensor("out", [P, T * HCK], F32, kind="ExternalOutput")

    # ---- internal DRAM ----
    xw_dram = nc.dram_tensor("xw_dram", [T * P, GC], F32)
    xin_b = nc.dram_tensor("xin_b", [P, T * P], F32R)
    xt_g = nc.dram_tensor("xt_g", [D, T * P], F32R, addr_space="Shared")
    atin_b = nc.dram_tensor("atin_b", [P, L * P], F32R)
    at_g = nc.dram_tensor("at_g", [H, L * P], F32R, addr_space="Shared")
    n_ag = t_steps * repeat  # one AllGather per step (h0's gather is step 0's)
    bin_d = nc.dram_tensor("bin_d", [n_ag, P, P + L], F32)
    bout_d = nc.dram_tensor("bout_d", [n_ag, NC * P, P + L], F32,
                            addr_space="Shared")

    rg = [list(range(NC))]

    with tile.TileContext(nc) as tc:
        # ---- gather the sharded xT / at feeds (one-time) ----
        nc.sync.dma_start(xin_b[:, :], xTs[:, :])
        nc.sync.dma_start(atin_b[:, :], ats[:, :])
        nc.gpsimd.collective_compute(
            "AllGather", mybir.AluOpType.bypass, replica_groups=rg,
            ins=[xin_b.ap()], outs=[xt_g.ap()])
        nc.gpsimd.collective_compute(
            "AllGather", mybir.AluOpType.bypass, replica_groups=rg,
            ins=[atin_b.ap()], outs=[at_g.ap()])

        # ================= static pools =================
        with tc.tile_pool(name="static", bufs=1) as sp, \
             tc.tile_pool(name="state", bufs=1) as statep:
            wh_sb = []
            for m in range(KC):
                t_ = sp.tile([P, GC], F32R, tag=f"wh{m}")
                nc.sync.dma_start(t_[:], wh[m * P:(m + 1) * P, :])
                wh_sb.append(t_)
            eye = sp.tile([P, P], F32R, tag="eye")
            nc.sync.dma_start(eye[:], eyeT[:, :])
            bias_sb = sp.tile([P, GC], F32R, tag="bias")
            nc.sync.dma_start(bias_sb[:], bia[:, :])
            asc_sb = sp.tile([P, L * HCK], F32, tag="asc")
            nc.sync.dma_start(asc_sb[:], asc[:, :])
            B_sb = [sp.tile([P, GC], F32R, tag=f"B{l}", name=f"B{l}")
                    for l in range(L)]

            c_st = statep.tile([P, HCK], F32, tag="c")

            # ============== prologue: B build ==============
            with tc.tile_pool(name="atp", bufs=1) as atp, \
                 tc.tile_pool(name="bps", bufs=4, space="PSUM") as bps:
                at_sb = []
                wat_sb = []
                for m in range(KC):
                    a_ = atp.tile([P, L * P], F32R, tag=f"at{m}")
                    nc.sync.dma_start(a_[:], at_g[m * P:(m + 1) * P, :])
                    at_sb.append(a_)
                    w_ = atp.tile([P, GC], F32R, tag=f"wat{m}")
                    nc.sync.dma_start(w_[:], wat[m * P:(m + 1) * P, :])
                    wat_sb.append(w_)
                for l in range(L):
                    bp = bps.tile([P, GC], F32, tag="bps")
                    for m in range(KC):
                        nc.tensor.matmul(
                            bp[:], at_sb[m][:, l * P:(l + 1) * P], wat_sb[m][:],
                            start=(m == 0), stop=(m == KC - 1),
                        )
                    nc.vector.tensor_copy(B_sb[l][:], bp[:])

            # ============== prologue: XW build ==============
            with tc.tile_pool(name="xtp", bufs=3) as xtp, \
                 tc.tile_pool(name="wxp", bufs=1) as wxp, \
                 tc.tile_pool(name="xwps", bufs=4, space="PSUM") as xwps:
                wx_sb = []
                for m in range(KC):
                    w_ = wxp.tile([P, GC], F32R, tag=f"wx{m}")
                    nc.sync.dma_start(w_[:], wx[m * P:(m + 1) * P, :])
                    wx_sb.append(w_)
                for j in range(t_steps):
                    xp = xwps.tile([P, GC], F32, tag="xwps")
                    nc.tensor.matmul(xp[:], eye[:], bias_sb[:], start=True, stop=False)
                    xt_ = xtp.tile([P, KC * P], F32R, tag="xt", name="xt")
                    nc.sync.dma_start(
                        xt_[:].rearrange("p (m c) -> p m c", m=KC),
                        xt_g.rearrange("(m p) c -> p m c", m=KC)
                        [:, :, j * P:(j + 1) * P])
                    for m in range(KC):
                        nc.tensor.matmul(xp[:], xt_[:, m * P:(m + 1) * P],
                                         wx_sb[m][:],
                                         start=False, stop=(m == KC - 1))
                    xs = xtp.tile([P, GC], F32, tag="xs", name="xs", bufs=3)
                    nc.vector.tensor_copy(xs[:], xp[:])
                    nc.sync.dma_start(xw_dram[j * P:(j + 1) * P, :], xs[:])

            # ============== h0/c0 init ==============
            # c0 = mean_l A[n,hc,l] = 2 * sum_l asc (asc pre-scaled by 1/32)
            with tc.tile_pool(name="initp", bufs=1) as initp:
                r_ = initp.tile([P, HCK], F32, tag="r")
                nc.vector.tensor_reduce(
                    r_[:],
                    asc_sb[:].rearrange("p (l c) -> p c l", l=L),
                    axis=mybir.AxisListType.X, op=mybir.AluOpType.add)
                nc.vector.tensor_scalar_mul(c_st[:], r_[:], 2.0)

            # ============== recurrent loop ==============
            with tc.tile_pool(name="hp", bufs=3) as hp, \
                 tc.tile_pool(name="htp", bufs=2) as htp, \
                 tc.tile_pool(name="dgp", bufs=8) as dgp, \
                 tc.tile_pool(name="xwsb", bufs=3) as xwsb, \
                 tc.tile_pool(name="smp", bufs=3) as smp, \
                 tc.tile_pool(name="gp", bufs=2) as gp, \
                 tc.tile_pool(name="prodp", bufs=2) as prodp, \
                 tc.tile_pool(name="aps", bufs=2, space="PSUM") as apsp, \
                 tc.tile_pool(name="tps", bufs=2, space="PSUM") as tpsp:

                h_t = hp.tile([P, HCK], F32R, tag="h")
                nc.vector.tensor_copy(h_t[:], c_st[:])  # h0 = c0

                for tg in range(t_steps * repeat):
                    t = tg % t_steps
                    # -- share: transpose h -> bounce; partial scores -> bounce
                    tp = tpsp.tile([P, P], F32R, tag="tp")
                    nc.tensor.transpose(tp[:], h_t[:], eye[:])
                    comb = htp.tile([P, P + L], F32, tag="comb", name="comb")
                    nc.scalar.copy(comb[:, 0:P], tp[:])

                    prod = prodp.tile([P, L * HCK], F32, tag="prod")
                    nc.vector.tensor_tensor(
                        prod[:],
                        h_t[:].unsqueeze(1).broadcast_to((P, L, HCK)),
                        asc_sb[:].rearrange("p (l c) -> p l c", l=L),
                        op=mybir.AluOpType.mult)
                    nc.vector.tensor_reduce(
                        comb[:, P:P + L],
                        prod[:].rearrange("p (l c) -> p l c", l=L),
                        axis=mybir.AxisListType.X, op=mybir.AluOpType.add)
                    nc.sync.dma_start(bin_d[tg], comb[:])

                    if use_cc:
                        nc.gpsimd.collective_compute(
                            "AllGather", mybir.AluOpType.bypass,
                            replica_groups=rg,
                            ins=[bin_d[tg]], outs=[bout_d[tg]])
                    else:
                        # timing-only variant: fake the gather with local
                        # copies (numerics wrong on 7/8 chunks)
                        for m in range(NC):
                            nc.sync.dma_start(
                                bout_d[tg, m * P:(m + 1) * P, :], bin_d[tg])

                    # -- bring back gathered h^T chunks + partials
                    hTall = htp.tile([P, NC * P], F32R, tag="hTall", name="hTall")
                    nc.sync.dma_start(
                        hTall[:].rearrange("p (j c) -> p j c", j=NC),
                        bout_d[tg].rearrange("(j n) c -> n j c", j=NC)[:, :, 0:P]
                        .bitcast(F32R))
                    parts = smp.tile([P, NC * L], F32, tag="parts")
                    nc.sync.dma_start(
                        parts[:].rearrange("p (j l) -> p j l", j=NC),
                        bout_d[tg].rearrange("(j n) c -> n j c", j=NC)[:, :, P:P + L])

                    # -- softmax over l
                    scr = smp.tile([P, L], F32, tag="scr")
                    nc.vector.tensor_reduce(
                        scr[:], parts[:].rearrange("p (j l) -> p l j", j=NC),
                        axis=mybir.AxisListType.X, op=mybir.AluOpType.add)
                    negm = smp.tile([P, 1], F32, tag="negm")
                    nc.vector.tensor_reduce(
                        negm[:], scr[:], axis=mybir.AxisListType.X,
                        op=mybir.AluOpType.max, negate=True)
                    ex = smp.tile([P, L], F32, tag="ex")
                    ssum = smp.tile([P, 1], F32, tag="ssum")
                    nc.scalar.activation(
                        ex[:], scr[:], mybir.ActivationFunctionType.Exp,
                        bias=negm[:], accum_out=ssum[:])
                    rcp = smp.tile([P, 1], F32, tag="rcp")
                    nc.vector.reciprocal(rcp[:], ssum[:])
                    wgt = smp.tile([P, L], F32, tag="wgt")
                    nc.vector.tensor_scalar_mul(wgt[:], ex[:], rcp[:])

                    # -- gates: a = XW_t + bias + h@Wh + sum_l w_l B_l
                    xw_t = xwsb.tile([P, GC], F32R, tag="xw")
                    nc.sync.dma_start(
                        xw_t[:], xw_dram[t * P:(t + 1) * P, :].bitcast(F32R))
                    ap_ = apsp.tile([P, GC], F32, tag="a")
                    nc.tensor.matmul(ap_[:], eye[:], xw_t[:], start=True, stop=False)
                    for m in range(NC):
                        nc.tensor.matmul(ap_[:], hTall[:, m * P:(m + 1) * P],
                                         wh_sb[m][:], start=False, stop=False)
                    for g_ in range(4):
                        dg = dgp.tile([P, 4 * P], F32R, tag="dg", name="dg")
                        nc.vector.scalar_tensor_tensor(
                            dg[:].rearrange("p (l c) -> p l c", l=4),
                            eye[:].unsqueeze(1).broadcast_to((P, 4, P)),
                            1.0,
                            wgt[:, 4 * g_:4 * g_ + 4].unsqueeze(2)
                            .broadcast_to((P, 4, P)),
                            op0=mybir.AluOpType.mult,
                            op1=mybir.AluOpType.mult)
                        for i_ in range(4):
                            l = 4 * g_ + i_
                            nc.tensor.matmul(
                                ap_[:], dg[:, i_ * P:(i_ + 1) * P], B_sb[l][:],
                                start=False, stop=(l == L - 1))

                    # -- activations + cell
                    sig = gp.tile([P, 3 * HCK], F32, tag="sig")
                    nc.scalar.activation(sig[:], ap_[:, 0:3 * HCK],
                                         mybir.ActivationFunctionType.Sigmoid)
                    tgate = gp.tile([P, HCK], F32, tag="tg", name="tgate")
                    nc.scalar.activation(tgate[:], ap_[:, 3 * HCK:GC],
                                         mybir.ActivationFunctionType.Tanh)
                    ig = gp.tile([P, HCK], F32, tag="ig")
                    nc.vector.tensor_mul(ig[:], sig[:, 0:HCK], tgate[:])
                    fc = gp.tile([P, HCK], F32, tag="fc")
                    nc.vector.tensor_mul(fc[:], sig[:, HCK:2 * HCK], c_st[:])
                    nc.vector.tensor_add(c_st[:], fc[:], ig[:])
                    th = gp.tile([P, HCK], F32, tag="th")
                    nc.scalar.activation(th[:], c_st[:],
                                         mybir.ActivationFunctionType.Tanh)
                    h_t = hp.tile([P, HCK], F32R, tag="h")
                    nc.vector.tensor_mul(h_t[:], sig[:, 2 * HCK:3 * HCK], th[:])

                    nc.sync.dma_start(
                        out[:, t * HCK:(t + 1) * HCK].bitcast(F32R), h_t[:])

    nc.compile()
    return nc


def _prep_inputs(x, A, Wx, Wh, Wattn, b):
    x = np.asarray(x, np.float32)
    A = np.asarray(A, np.float32)
    Wx = np.asarray(Wx, np.float32)
    Wh = np.asarray(Wh, np.float32)
    Wattn = np.asarray(Wattn, np.float32)
    b = np.asarray(b, np.float32)
    A_flat = A.reshape(N, H, L)

    # x transposed: [d, t*128+n]
    xT = np.ascontiguousarray(x.transpose(2, 1, 0).reshape(D, T * N))
    # A^T for B build: [h, l*128+n]
    at = np.ascontiguousarray(A_flat.transpose(1, 2, 0).reshape(H, L * N))
    eye = np.eye(P, dtype=np.float32)

    in_maps = []
    for k in range(NC):
        cols = np.concatenate(
            [g * H + np.arange(k * HCK, (k + 1) * HCK) for g in range(4)])
        asc_k = np.ascontiguousarray(
            A_flat[:, k * HCK:(k + 1) * HCK, :].transpose(0, 2, 1)
            .reshape(N, L * HCK) / np.sqrt(np.float32(H)))
        in_maps.append({
            "xTs": np.ascontiguousarray(xT[k * P:(k + 1) * P, :]),
            "wx": np.ascontiguousarray(Wx[:, cols]),
            "wh": np.ascontiguousarray(Wh[:, cols]),
            "wat": np.ascontiguousarray(Wattn[:, cols]),
            "bia": np.ascontiguousarray(np.broadcast_to(b[cols], (P, GC))),
            "asc": asc_k,
            "ats": np.ascontiguousarray(at[k * P:(k + 1) * P, :]),
            "eyeT": eye,
        })
    return in_maps


def kernel(x, A, Wx, Wh, Wattn, b, t_steps=T, use_cc=True, repeat=1):
    key = (t_steps, use_cc, repeat)
    if key not in _cache:
        _cache[key] = _build(t_steps, use_cc, repeat)
    nc = _cache[key]
    in_maps = _prep_inputs(x, A, Wx, Wh, Wattn, b)
    res = run_bass_kernel_spmd(nc, in_maps, core_ids=list(range(NC)), trace=False)
    global LAST_EXEC_NS
    LAST_EXEC_NS = res.exec_time_ns
    outp = np.empty((N, t_steps, H), np.float32)
    for k in range(NC):
        o = res.results[k]["out"].reshape(N, T, HCK)
        outp[:, :, k * HCK:(k + 1) * HCK] = o[:, :t_steps, :]
    return outp


LAST_EXEC_NS = None



# revision 2
# speedup vs baseline: 57.9689x; 57.9689x over previous
"""AttentionLSTM Trainium2 kernel v2: 8-core DATA-parallel (16 samples/core),
zero per-step collectives, recurrent body in a For_i hardware loop.

Cost model for this (axon/fake_nrt) stack, measured by microbenchmark:
  - per STATIC instruction per call: ~24 us  -> keep the NEFF tiny (For_i)
  - per collective: ~400 us                  -> only 3 prologue AllGathers
  - dynamic loop iterations: ~2 us/instr serialized, less when pipelined

Design:
  - core k owns samples n in [16k, 16k+16). Weights are sharded on the host
    (row-chunks), AllGathered once on device, held in SBUF as fp16.
  - prologue GEMM: xw = x@Wx + b for all T in a For_i over 8 row-tiles.
  - per step (For_i over t): attention in an (n,m) partition layout
    (p = n*8+m owns A[n, m*128:(m+1)*128, :]), selector matmuls for the
    cross-chunk score sum and softmax broadcast, PE transposes to build the
    fp16 lhsT tiles of h and attn, 136 PSUM-accumulating matmuls for
    a = xw_t + h@Wh + attn@(32*Wattn), fused sigmoid/tanh, cell update.
  - state layout [32, 512]: partition q*16+n holds h[n, q*512:(q+1)*512].
  - output written per step as [32, 512] -> out_d[:, t*512:...]; host fixes
    the layout.
"""

import os
import sys

sys.path.insert(0, "/opt/trn_rl_repo")

import numpy as np

import concourse.bass as bass
import concourse.tile as tile
from concourse import bacc, mybir
from concourse.bass import ds, ts
from concourse.bass_utils import run_bass_kernel_spmd

N, T, D, H = 128, 64, 1024, 1024
L = 16
NC = 8
NS = N // NC           # samples per core = 16
P = 128
G = 4 * H              # gate cols = 4096
KC = 8                 # 128-row contraction chunks in D/H

F32 = mybir.dt.float32
F32R = mybir.dt.float32r
F16 = mybir.dt.float16

_cache = {}


def _build(t_steps: int, use_cc: bool = True, repeat: int = 1, probe: str = ""):
    nc = bacc.Bacc(
        "TRN2",
        target_bir_lowering=False,
        debug=False,
        enable_asserts=False,
        num_devices=NC,
    )

    # ---- kernel I/O ----
    xs = nc.dram_tensor("xs", [P, KC * NS * T], F16, kind="ExternalInput")
    wxs = nc.dram_tensor("wxs", [P, G], F16, kind="ExternalInput")
    whs = nc.dram_tensor("whs", [P, G], F16, kind="ExternalInput")
    was = nc.dram_tensor("was", [P, G], F16, kind="ExternalInput")
    bia = nc.dram_tensor("bia", [1, G], F16, kind="ExternalInput")
    ascs = nc.dram_tensor("ascs", [P, L * P], F16, kind="ExternalInput")
    c0s = nc.dram_tensor("c0s", [NS, H], F32, kind="ExternalInput")
    selM = nc.dram_tensor("selM", [P, NS], F16, kind="ExternalInput")
    selMT = nc.dram_tensor("selMT", [NS, P], F16, kind="ExternalInput")
    eyeT = nc.dram_tensor("eyeT", [P, P], F32R, kind="ExternalInput")
    eye16 = nc.dram_tensor("eye16", [NS, NS], F16, kind="ExternalInput")
    ones1 = nc.dram_tensor("ones1", [1, P], F16, kind="ExternalInput")
    out = nc.dram_tensor("out", [NS, T * H], F16, kind="ExternalOutput")

    # ---- internal DRAM ----
    wxi = nc.dram_tensor("wxi", [P, G], F16)
    whi = nc.dram_tensor("whi", [P, G], F16)
    wai = nc.dram_tensor("wai", [P, G], F16)
    wx_g = nc.dram_tensor("wx_g", [D, G], F16, addr_space="Shared")
    wh_g = nc.dram_tensor("wh_g", [H, G], F16, addr_space="Shared")
    wa_g = nc.dram_tensor("wa_g", [H, G], F16, addr_space="Shared")
    xw_dram = nc.dram_tensor("xw_dram", [NS * T, G], F16)
    h_d = nc.dram_tensor("h_d", [NS, H], F32R)

    rg = [list(range(NC))]

    with tile.TileContext(nc) as tc:
        # ---- one-time weight AllGathers (rank k supplies rows 128k..) ----
        nc.sync.dma_start(wxi[:, :], wxs[:, :])
        nc.sync.dma_start(whi[:, :], whs[:, :])
        nc.sync.dma_start(wai[:, :], was[:, :])
        if use_cc:
            nc.gpsimd.collective_compute(
                "AllGather", mybir.AluOpType.bypass, replica_groups=rg,
                ins=[wxi.ap()], outs=[wx_g.ap()])
            nc.gpsimd.collective_compute(
                "AllGather", mybir.AluOpType.bypass, replica_groups=rg,
                ins=[whi.ap()], outs=[wh_g.ap()])
            nc.gpsimd.collective_compute(
                "AllGather", mybir.AluOpType.bypass, replica_groups=rg,
                ins=[wai.ap()], outs=[wa_g.ap()])
        else:
            for m in range(KC):
                nc.sync.dma_start(wx_g[m * P:(m + 1) * P, :], wxi[:, :])
                nc.sync.dma_start(wh_g[m * P:(m + 1) * P, :], whi[:, :])
                nc.sync.dma_start(wa_g[m * P:(m + 1) * P, :], wai[:, :])

        with tc.tile_pool(name="static", bufs=1) as sp, \
             tc.tile_pool(name="state", bufs=1) as statep:
            # ---- persistent SBUF ----
            wh_sb = []
            wa_sb = []
            for m in range(KC):
                t_ = sp.tile([P, G], F16, tag=f"wh{m}")
                nc.sync.dma_start(t_[:], wh_g[m * P:(m + 1) * P, :])
                wh_sb.append(t_)
            for m in range(KC):
                t_ = sp.tile([P, G], F16, tag=f"wa{m}")
                nc.sync.dma_start(t_[:], wa_g[m * P:(m + 1) * P, :])
                wa_sb.append(t_)
            asc_sb = sp.tile([P, L * P], F32, tag="asc")
            eye = sp.tile([P, P], F32R, tag="eye")
            nc.sync.dma_start(eye[:], eyeT[:, :])
            e16 = sp.tile([NS, NS], F16, tag="e16")
            nc.sync.dma_start(e16[:], eye16[:, :])
            sM = sp.tile([P, NS], F16, tag="sM")
            nc.sync.dma_start(sM[:], selM[:, :])
            sMT = sp.tile([NS, P], F16, tag="sMT")
            nc.sync.dma_start(sMT[:], selMT[:, :])
            on1 = sp.tile([1, P], F16, tag="on1")
            nc.sync.dma_start(on1[:], ones1[:, :])
            bia_sb = sp.tile([1, G], F16, tag="bia")
            nc.sync.dma_start(bia_sb[:], bia[:, :])

            c_st = statep.tile([NS, H], F32, tag="c")
            h_st = statep.tile([NS, H], F32R, tag="h")
            nc.sync.dma_start(c_st[:], c0s[:, :])
            nc.vector.tensor_copy(h_st[:], c_st[:])
            nc.sync.dma_start(h_d[:, :], c0s.ap().bitcast(F32R))

            # ============ prologue: xw = x@Wx + b ============
            with tc.tile_pool(name="xsp", bufs=2) as xsp, \
                 tc.tile_pool(name="wxp", bufs=2) as wxp, \
                 tc.tile_pool(name="xwo", bufs=2) as xwop, \
                 tc.tile_pool(name="xwps", bufs=1, space="PSUM") as xwps:
                asc16 = xsp.tile([P, L * P], F16, tag="asc16")
                nc.sync.dma_start(asc16[:], ascs[:, :])
                nc.vector.tensor_copy(asc_sb[:], asc16[:])
                xw_ps = xwps.tile([P, G], F32, tag="xwps")
                with tc.For_i(0, KC, 1) as rt:
                    # x-block for this row-tile: [128 d, (m), 128 r]
                    xsc = xsp.tile([P, KC * P], F16, tag="xsc")
                    nc.sync.dma_start(
                        xsc[:].rearrange("p (m r) -> p m r", m=KC),
                        xs.rearrange("p (m r) -> p m r", m=KC)
                        [:, :, ds(rt * P, P)])
                    for gt in range(KC):
                        nc.tensor.matmul(
                            xw_ps[:, gt * 512:(gt + 1) * 512], on1[:],
                            bia_sb[:, gt * 512:(gt + 1) * 512],
                            start=True, stop=False)
                    for m in range(KC):
                        wxc = wxp.tile([P, G], F16, tag="wxc")
                        nc.sync.dma_start(wxc[:], wx_g[m * P:(m + 1) * P, :])
                        for gt in range(KC):
                            nc.tensor.matmul(
                                xw_ps[:, gt * 512:(gt + 1) * 512],
                                xsc[:, m * P:(m + 1) * P],
                                wxc[:, gt * 512:(gt + 1) * 512],
                                start=False, stop=(m == KC - 1))
                    xw_o = xwop.tile([P, G], F16, tag="xwo")
                    nc.vector.tensor_copy(xw_o[:], xw_ps[:])
                    nc.sync.dma_start(xw_dram[ds(rt * P, P), :], xw_o[:])

            # ============ recurrent loop ============
            with tc.tile_pool(name="hTp", bufs=1) as hTp, \
                 tc.tile_pool(name="nmp", bufs=1) as nmp, \
                 tc.tile_pool(name="smp", bufs=1) as smp, \
                 tc.tile_pool(name="xwsb", bufs=1) as xwsb, \
                 tc.tile_pool(name="gp", bufs=1) as gp, \
                 tc.tile_pool(name="aps", bufs=1, space="PSUM") as apsp, \
                 tc.tile_pool(name="tps", bufs=1, space="PSUM") as tpsp:

                n_mm = 0 if "nomm" in probe else KC

                def half_gates(a_ps, xw_t, hT, atTr, ph):
                    # columns [ph*2048, (ph+1)*2048) of the (host-permuted)
                    # gate matrix: a = xw + h@Wh + (attn/32)@(32*Wattn)
                    for g2 in range(4):
                        gt = ph * 4 + g2
                        o_ = a_ps[:, g2 * 512:(g2 + 1) * 512]
                        nc.tensor.matmul(
                            o_, e16[:], xw_t[:, gt * 512:(gt + 1) * 512],
                            start=True, stop=(n_mm == 0))
                        for m in range(n_mm):
                            nc.tensor.matmul(
                                o_, hT[:, m * NS:(m + 1) * NS],
                                wh_sb[m][:, gt * 512:(gt + 1) * 512],
                                start=False, stop=False)
                        for m in range(n_mm):
                            nc.tensor.matmul(
                                o_, atTr[:, m, :],
                                wa_sb[m][:, gt * 512:(gt + 1) * 512],
                                start=False, stop=(m == KC - 1))

                def body(i):
                    # -- xw prefetch (depends only on i)
                    xw_t = xwsb.tile([NS, G], F16, tag="xw")
                    if "nodma" not in probe:
                        nc.sync.dma_start(xw_t[:], xw_dram[ds(i * NS, NS), :])
                    else:
                        nc.any.memzero(xw_t)

                    # -- hT: 8 transposes [16,128] -> [128,16] into one bank
                    hT_ps = tpsp.tile([P, P], F32R, tag="hT")
                    if "notr" in probe:
                        nc.any.memzero(hT_ps)
                    for m in range(0 if "notr" in probe else KC):
                        nc.tensor.transpose(
                            hT_ps[:, m * NS:(m + 1) * NS],
                            h_st[:, m * P:(m + 1) * P],
                            eye[0:NS, 0:NS])
                    hT = hTp.tile([P, P], F16, tag="hTf")
                    nc.vector.tensor_copy(hT[:], hT_ps[:])

                    # -- h_nm [128 (n,m), 128 hc] from the h_d DRAM copy
                    h_nm = nmp.tile([P, P], F32R, tag="hnm")
                    if "nodma" not in probe:
                        nc.sync.dma_start(
                            h_nm[:],
                            h_d.rearrange("n (m c) -> (n m) c", m=KC))
                    else:
                        nc.any.memzero(h_nm)

                    # -- scores: per-partition partial over 128 h-cols
                    novec = "novec" in probe
                    prod = smp.tile([P, L * P], F32, tag="prod")
                    if not novec:
                        nc.vector.tensor_tensor(
                        prod[:],
                        h_nm[:].unsqueeze(1).broadcast_to((P, L, P)),
                        asc_sb[:].rearrange("p (l c) -> p l c", l=L),
                            op=mybir.AluOpType.mult)
                    sp_ = smp.tile([P, L], F16, tag="sp")
                    if not novec:
                        with nc.allow_low_precision(
                                reason="DVE ALU accumulates fp32; f16 store"):
                            nc.vector.tensor_reduce(
                                sp_[:],
                                prod[:].rearrange("p (l c) -> p l c", l=L),
                                axis=mybir.AxisListType.X,
                                op=mybir.AluOpType.add)
                    else:
                        nc.vector.tensor_copy(
                            sp_[:], asc_sb[:, 0:L].bitcast(F32))
                    # -- cross-chunk sum: sc[n, l] = sum_m sp[(n,m), l]
                    sc_ps = tpsp.tile([NS, L], F32, tag="sc")
                    nc.tensor.matmul(sc_ps[:], sM[:], sp_[:],
                                     start=True, stop=True)
                    # -- softmax over l (no max-subtract: |scores| <= 32)
                    ex = smp.tile([NS, L], F32, tag="ex")
                    ssum = smp.tile([NS, 1], F32, tag="ssum")
                    nc.scalar.activation(
                        ex[:], sc_ps[:], mybir.ActivationFunctionType.Exp,
                        accum_out=ssum[:])
                    rcp = smp.tile([NS, 1], F32, tag="rcp")
                    nc.vector.reciprocal(rcp[:], ssum[:])
                    wgt = smp.tile([NS, L], F16, tag="wgt")
                    nc.vector.tensor_scalar_mul(wgt[:], ex[:], rcp[:])
                    # -- broadcast w back to (n,m) partitions
                    wb_ps = tpsp.tile([P, L], F32, tag="wb")
                    nc.tensor.matmul(wb_ps[:], sMT[:], wgt[:],
                                     start=True, stop=True)

                    # -- attn_nm[(n,m), hc] = sum_l asc * w  (= attn/32)
                    prod2 = smp.tile([P, L * P], F32, tag="prod")
                    attn_nm = nmp.tile([P, P], F32R, tag="attn")
                    if not novec:
                        nc.vector.tensor_tensor(
                            prod2[:],
                            wb_ps[:].unsqueeze(2).broadcast_to((P, L, P)),
                            asc_sb[:].rearrange("p (l c) -> p l c", l=L),
                            op=mybir.AluOpType.mult)
                        with nc.allow_low_precision(reason="f32r bits f32"):
                            nc.vector.tensor_reduce(
                                attn_nm[:],
                                prod2[:].rearrange("p (l c) -> p c l", l=L),
                                axis=mybir.AxisListType.X,
                                op=mybir.AluOpType.add)
                    else:
                        nc.vector.tensor_copy(
                            attn_nm[:], h_nm[:])
                    # -- atT[hc, (n,m)]: one [128,128] transpose
                    atT_ps = tpsp.tile([P, P], F32R, tag="atT")
                    if "notr" in probe:
                        nc.any.memzero(atT_ps)
                    else:
                        nc.tensor.transpose(atT_ps[:], attn_nm[:], eye[:])
                    atT = hTp.tile([P, P], F16, tag="atTf")
                    nc.vector.tensor_copy(atT[:], atT_ps[:])
                    atTr = atT[:].rearrange("p (n m) -> p m n", m=KC)

                    # -- phase 1: gates i and tanh-gate g (permuted cols)
                    a_ps = apsp.tile([NS, 2048], F32, tag="a")
                    half_gates(a_ps, xw_t, hT, atTr, 0)
                    sig_i = gp.tile([NS, H], F32, tag="sig_i")
                    nc.scalar.activation(sig_i[:], a_ps[:, 0:H],
                                         mybir.ActivationFunctionType.Sigmoid)
                    tg = gp.tile([NS, H], F32, tag="tg")
                    nc.scalar.activation(tg[:], a_ps[:, H:2 * H],
                                         mybir.ActivationFunctionType.Tanh)
                    ig = gp.tile([NS, H], F32, tag="ig")
                    nc.vector.tensor_mul(ig[:], sig_i[:], tg[:])

                    # -- phase 2: gates f and o (same psum buffer, rotated)
                    a_ps2 = apsp.tile([NS, 2048], F32, tag="a")
                    half_gates(a_ps2, xw_t, hT, atTr, 1)
                    sig_fo = gp.tile([NS, 2 * H], F32, tag="sig_fo")
                    nc.scalar.activation(sig_fo[:], a_ps2[:],
                                         mybir.ActivationFunctionType.Sigmoid)
                    fc = gp.tile([NS, H], F32, tag="fc")
                    nc.vector.tensor_mul(fc[:], sig_fo[:, 0:H], c_st[:])
                    nc.vector.tensor_add(c_st[:], fc[:], ig[:])
                    th = gp.tile([NS, H], F32, tag="th")
                    nc.scalar.activation(th[:], c_st[:],
                                         mybir.ActivationFunctionType.Tanh)
                    nc.vector.tensor_mul(h_st[:], sig_fo[:, H:2 * H], th[:])

                    h16 = gp.tile([NS, H], F16, tag="h16")
                    nc.vector.tensor_copy(h16[:], h_st[:])
                    if "nodma" not in probe:
                        nc.sync.dma_start(out[:, ds(i * H, H)], h16[:])
                        nc.sync.dma_start(h_d[:, :], h_st[:])

                if repeat == 1:
                    with tc.For_i(0, t_steps, 1) as i:
                        body(i)
                else:
                    with tc.For_i(0, repeat, 1) as r_i:
                        with tc.For_i(0, t_steps, 1) as i:
                            body(i)

    nc.compile()
    return nc


def _prep_inputs(x, A, Wx, Wh, Wattn, b):
    x = np.asarray(x, np.float32)
    A = np.asarray(A, np.float32)
    Wx = np.asarray(Wx, np.float32)
    Wh = np.asarray(Wh, np.float32)
    Wattn = np.asarray(Wattn, np.float32)
    b = np.asarray(b, np.float32)
    A_flat = A.reshape(N, H, L)

    eye = np.eye(P, dtype=np.float32)
    e16 = np.eye(NS, dtype=np.float16)
    on1 = np.ones((1, P), dtype=np.float16)
    selM = np.zeros((P, NS), dtype=np.float16)
    for n in range(NS):
        selM[n * KC:(n + 1) * KC, n] = 1.0
    selMT = np.ascontiguousarray(selM.T)
    # gate-column permutation: [i, g, f, o] so phase1=(i,g), phase2=(f,o)
    perm = np.concatenate([np.arange(0, H), np.arange(3 * H, 4 * H),
                           np.arange(H, 2 * H), np.arange(2 * H, 3 * H)])
    bia = b[perm].reshape(1, G).astype(np.float16)
    wa32 = (Wattn[:, perm] * 32.0).astype(np.float16)
    wx16 = Wx[:, perm].astype(np.float16)
    wh16 = Wh[:, perm].astype(np.float16)
    c0 = A.mean(axis=(2, 3)).astype(np.float32)  # [N, H]

    in_maps = []
    for k in range(NC):
        sl = slice(k * NS, (k + 1) * NS)
        # xs[p, m*NS*T + t*NS + n] = x[nk+n, t, m*128+p]
        xsl = x[sl].transpose(2, 1, 0).astype(np.float16)   # [D, T, NS]
        xsl = xsl.reshape(KC, P, T * NS).transpose(1, 0, 2).reshape(
            P, KC * T * NS)
        # ascs[n*8+m, l*128+hc] = A[nk+n, m*128+hc, l] / 32
        asl = A_flat[sl].reshape(NS, KC, P, L).transpose(0, 1, 3, 2)
        asl = (asl / 32.0).reshape(P, L * P).astype(np.float16)
        in_maps.append({
            "c0s": np.ascontiguousarray(c0[sl]),
            "xs": np.ascontiguousarray(xsl),
            "wxs": np.ascontiguousarray(wx16[k * P:(k + 1) * P, :]),
            "whs": np.ascontiguousarray(wh16[k * P:(k + 1) * P, :]),
            "was": np.ascontiguousarray(wa32[k * P:(k + 1) * P, :]),
            "bia": bia,
            "ascs": np.ascontiguousarray(asl),
            "selM": selM,
            "selMT": selMT,
            "eyeT": eye,
            "eye16": e16,
            "ones1": on1,
        })
    return in_maps


_prep_cache = {}


def _prep_key(*arrs):
    parts = []
    for a in arrs:
        a = np.asarray(a)
        flat = a.reshape(-1)
        stride = max(1, flat.size // 2048)
        parts.append((a.shape, str(a.dtype), flat[::stride].tobytes()))
    return hash(tuple(parts))


def kernel(x, A, Wx, Wh, Wattn, b, t_steps=T, use_cc=True, repeat=1,
           probe=""):
    key = (t_steps, use_cc, repeat, probe)
    if key not in _cache:
        _cache[key] = _build(t_steps, use_cc, repeat, probe)
    nc = _cache[key]
    pkey = _prep_key(x, A, Wx, Wh, Wattn, b)
    if pkey not in _prep_cache:
        _prep_cache.clear()
        _prep_cache[pkey] = _prep_inputs(x, A, Wx, Wh, Wattn, b)
    in_maps = _prep_cache[pkey]
    res = run_bass_kernel_spmd(nc, in_maps, core_ids=list(range(NC)),
                               trace=False)
    global LAST_EXEC_NS
    LAST_EXEC_NS = res.exec_time_ns
    outp = np.empty((N, t_steps, H), np.float32)
    for k in range(NC):
        o = res.results[k]["out"].reshape(NS, T, H)
        outp[k * NS:(k + 1) * NS] = o[:, :t_steps, :].astype(np.float32)
    return outp


LAST_EXEC_NS = None
